# revision 22
# baseline (speedup 1.0000x reference)
"""NanoVLM GQA attention block on 8 Trainium2 NeuronCores.

Sharding: core c = 4*b + g handles batch b (of 2) and head-group g (of 4):
8 q-heads (global 8g..8g+8) and 2 kv-heads (2g, 2g+1). Each core computes a
partial output restricted to its heads' columns of Wo; the host sums the 4
partials per batch (the tensor-parallel reduce, done host-side).

v4: 16-bit datapath (fp16 for x/weights/q/k, bf16 downstream of the ACT
engine), XBAR DMA transposes, causal mask as post-exp tril multiply.

All engine queues are in-order, so emission order is the schedule. The
attention softmax (ACT exp, ~0.7us per tile) paces attention pairs while
the PE only has ~40% utilization there; conversely projection/out-proj are
pure PE. v4 therefore interleaves at matmul-chunk granularity: a filler
queue holds the PE-heavy work (th1 projection during attention half 0,
out-proj of half 0 during attention half 1) and is drained a chunk at a
time inside each pair's kc loop, so the PE queue always holds exp-independent
work behind the attention matmuls. PSUM: 2 rotating proj banks + 4 rotating
score/out-proj banks + 2 pinned y-accumulators. The per-(h,kc) exp is a
single ACT instruction (vq=0 bias); mixed vision/text q-ranges get a cheap
per-partition DVE fixup multiply instead of a second exp.
"""

import os
import sys

sys.path.insert(0, "/opt/trn_rl_repo")

import numpy as np
import ml_dtypes

import concourse.bacc as bacc
import concourse.mybir as mybir
import concourse.tile as tile
from concourse.bass_utils import run_bass_kernel_spmd

F32 = mybir.dt.float32
F16 = mybir.dt.float16
BF16 = mybir.dt.bfloat16
AF = mybir.ActivationFunctionType
ALU = mybir.AluOpType
BF = ml_dtypes.bfloat16

B, T, C = 2, 1024, 2048
NH, NKV, HD = 32, 8, 64
QH, KVH = 8, 2          # per-core q-heads / kv-heads
NTB = T // 128          # 8 t-blocks
NCORES = 8
NEG = -1e30


def _fixup_runs(qtile_vq, s, ql):
    """[c0,c1) column runs with vq==1 inside [ql,512) of an s-half tile."""
    runs, c = [], ql
    while c < 512:
        if qtile_vq[s * 4 + c // 128]:
            ce = c
            while ce < 512 and qtile_vq[s * 4 + ce // 128]:
                ce += 128
            runs.append((c, ce))
            c = ce
        else:
            c += 128
    return runs


def build_program(qtile_vq):
    """qtile_vq: per 128-token q-tile, the is_vision value (0/1), len 8."""
    nc = bacc.Bacc("TRN2", target_bir_lowering=False, debug=False,
                   num_devices=NCORES)

    xT_d = nc.dram_tensor("xT", [C, T], F16, kind="ExternalInput").ap()
    wq_d = nc.dram_tensor("wqT", [C, 512], F16, kind="ExternalInput").ap()
    wkv_d = nc.dram_tensor("wkvT", [C, 256], F16, kind="ExternalInput").ap()
    wo_d = nc.dram_tensor("woT", [512, C], BF16, kind="ExternalInput").ap()
    cos_d = nc.dram_tensor("cosT", [128, T], F32, kind="ExternalInput").ap()
    sin_d = nc.dram_tensor("sinT", [128, T], F32, kind="ExternalInput").ap()
    btab_d = nc.dram_tensor("btab", [128, 128], F32, kind="ExternalInput").ap()
    ftab_d = nc.dram_tensor("ftab", [128, 16], F32, kind="ExternalInput").ap()
    tril_d = nc.dram_tensor("trilT", [128, 128], BF16, kind="ExternalInput").ap()
    ones_d = nc.dram_tensor("ones16", [128, 16], BF16, kind="ExternalInput").ap()
    out_d = nc.dram_tensor("out", [T, C], BF16, kind="ExternalOutput").ap()

    with tile.TileContext(nc) as tc:
        cp_cm = tc.tile_pool(name="const", bufs=1)
        cp = cp_cm.__enter__()
        qTp = [cp.tile([128, T], F16, tag=f"qTp{p}", name=f"qTp{p}")
               for p in range(4)]
        kTp = cp.tile([128, T], F16, tag="kTp")
        kTs = cp.tile([128, T], F16, tag="kTs")  # kv heads swapped
        # [t-part, (tb,j) block, 66]: cols 0:64 v data, col 64 ones
        vAall = cp.tile([128, 16, 66], BF16, tag="vAall")
        vT = [cp.tile([128, 4, 64], BF16, tag=f"vT{i}", name=f"vT{i}")
              for i in range(4)]
        # y_aug evac pads: rows 0:65 used, 65:80 zero filler for the XBAR
        # transpose; declared [96,T] so the filler memset starts at the
        # 32-aligned partition 64 (row 64 is rewritten by every evac)
        yap = [cp.tile([96, T], BF16, tag=f"yap{i}", name=f"yap{i}")
               for i in range(4)]
        yT = [cp.tile([128, T], BF16, tag=f"yT{p}", name=f"yTt{p}")
              for p in range(4)]
        btab = cp.tile([128, 128], F32, tag="btab")
        ftab = cp.tile([128, 16], F32, tag="ftab")
        trilT = cp.tile([128, 128], BF16, tag="trilT")
        cosT = cp.tile([128, T], F32, tag="cosT")
        sinT = cp.tile([128, T], F32, tag="sinT")

        # --------- pools ---------------------------------------------------
        p1w_cm = tc.tile_pool(name="p1w", bufs=1, side="right")
        p1w = p1w_cm.__enter__()
        p1t_cm = tc.tile_pool(name="p1t", bufs=2, side="right")
        p1t = p1t_cm.__enter__()
        p1x_cm = tc.tile_pool(name="p1x", bufs=1, side="right")
        p1x = p1x_cm.__enter__()
        ps_cm = tc.tile_pool(name="ps", bufs=1, space="PSUM")
        ps = ps_cm.__enter__()
        ptp_cm = tc.tile_pool(name="ptp", bufs=8)
        ptp = ptp_cm.__enter__()
        p2t_cm = tc.tile_pool(name="p2t", bufs=4)
        p2t = p2t_cm.__enter__()
        ost_cm = tc.tile_pool(name="ost", bufs=2, side="right")
        ost = ost_cm.__enter__()

        # --------- input DMAs (queue order matters per engine) -------------
        xa, xb, wqs, wkvs = [], [], [], []
        for i in range(16):
            kt = p1w.tile([128, 256], F16, tag=f"wkv{i}", name=f"wkv{i}")
            nc.scalar.dma_start(kt[:], wkv_d[i * 128:(i + 1) * 128, :])
            wkvs.append(kt)
            xt = p1x.tile([128, 512], F16, tag=f"xa{i}", name=f"xa{i}")
            nc.sync.dma_start(xt[:], xT_d[i * 128:(i + 1) * 128, 0:512])
            xa.append(xt)
            wt = p1w.tile([128, 512], F16, tag=f"wq{i}", name=f"wq{i}")
            nc.gpsimd.dma_start(wt[:], wq_d[i * 128:(i + 1) * 128, :])
            wqs.append(wt)
        nc.scalar.dma_start(cosT[:], cos_d)
        nc.scalar.dma_start(sinT[:], sin_d)
        nc.scalar.dma_start(btab[:], btab_d)
        nc.scalar.dma_start(ftab[:], ftab_d)
        nc.scalar.dma_start(trilT[:], tril_d)
        wo = []
        for p in range(4):
            t = p1w.tile([128, C], BF16, tag=f"wo{p}", name=f"wo{p}")
            nc.scalar.dma_start(t[:], wo_d[p * 128:(p + 1) * 128, :])
            wo.append(t)
        for i in range(16):
            xt = p1x.tile([128, 512], F16, tag=f"xb{i}", name=f"xb{i}")
            nc.sync.dma_start(xt[:], xT_d[i * 128:(i + 1) * 128, 512:1024])
            xb.append(xt)
        # late gpsimd setup (after weight DMA issues): ones col + yap floor
        nc.gpsimd.dma_start(vAall[:, :, 64:65], ones_d)
        for i in range(4):
            nc.gpsimd.memset(yap[i][64:96, :], 0.0)

        def rope_blk(pp, blk, th):
            """pp: [128,512] psum with d-major proj; writes qTp/kTp th-slice."""
            tsl = slice(th * 512, (th + 1) * 512)
            dstT = qTp[blk] if blk < 4 else kTp
            # cos is 32-periodic along d and sinT carries the rotate-half
            # sign, so q*cos + rot(q)*sin == u - swap32(v) with u = q*cos,
            # v = q*sinT — both straight PSUM reads, no ACT evacuation
            u = p1t.tile([128, 512], F32, tag="t1", name="u")
            nc.vector.tensor_mul(u[:], pp[:], cosT[:, tsl])
            v = p1t.tile([128, 512], BF16, tag="t2", name="v")
            nc.vector.tensor_mul(v[:], pp[:], sinT[:, tsl])
            vs = p1t.tile([128, 512], BF16, tag="rot", name="vs")
            for q0 in (0, 64):
                nc.gpsimd.dma_start(vs[q0:q0 + 32, :], v[q0 + 32:q0 + 64, :])
                nc.gpsimd.dma_start(vs[q0 + 32:q0 + 64, :], v[q0:q0 + 32, :])
            nc.vector.tensor_sub(dstT[:, tsl], u[:], vs[:])
            if blk == 4:
                nc.gpsimd.dma_start(kTs[0:64, tsl], kTp[64:128, tsl])
                nc.gpsimd.dma_start(kTs[64:128, tsl], kTp[0:64, tsl])

        def v_path(th, pp):
            """evac v proj psum + transpose to t-major vAall slices."""
            vsb = p1t.tile([128, 512], BF16, tag="vsb", name="vsb")
            nc.scalar.copy(vsb[:], pp[:])
            for j in range(KVH):
                vt = vT[th * 2 + j]
                nc.sync.dma_start_transpose(vt[:], vsb[j * 64:(j + 1) * 64, :])
                for qb in range(4):
                    tb = th * 4 + qb
                    nc.vector.tensor_copy(vAall[:, tb * 2 + j, 0:64],
                                          vt[:, qb, :])

        def proj_chunks(th, filler):
            """queue one th's projection as 4-ci chunks on the filler."""
            xs = xa if th == 0 else xb
            for blk in (4, 0, 1, 2, 3, 5):
                state = {}

                def mk(blk, c0, state):
                    def chunk():
                        if c0 == 0:
                            state["pp"] = ps.tile(
                                [128, 512], F32, tag="pp", bufs=2,
                                name=f"pp{th}_{blk}")
                        pp = state["pp"]
                        for ci in range(c0, c0 + 4):
                            if blk < 4:
                                lhsT = wqs[ci][:, blk * 128:(blk + 1) * 128]
                            else:
                                lhsT = wkvs[ci][:, (blk - 4) * 128:(blk - 3) * 128]
                            nc.tensor.matmul(pp[:], lhsT, xs[ci][:],
                                             start=(ci == 0), stop=(ci == 15))
                        if c0 == 12:
                            if blk == 5:
                                v_path(th, pp)
                            else:
                                rope_blk(pp, blk, th)
                    return chunk

                for c0 in (0, 4, 8, 12):
                    filler.append(mk(blk, c0, state))

        def outproj_chunks(tb, filler):
            """queue one out-proj t-block as per-n chunks on the filler."""
            trow = slice(tb * 128, (tb + 1) * 128)
            state = {}

            def mk(n, state):
                def chunk():
                    if n == 0:
                        state["oe"] = ost.tile([128, C], BF16, tag="oe",
                                               name="oe")
                    oe = state["oe"]
                    op = ps.tile([128, 512], F32, tag="pp", bufs=2, name="op")
                    for p in range(4):
                        nc.tensor.matmul(
                            op[:], yT[p][:, trow],
                            wo[p][:, n * 512:(n + 1) * 512],
                            start=(p == 0), stop=(p == 3))
                    nc.vector.tensor_copy(oe[:, n * 512:(n + 1) * 512], op[:])
                    if tb < 4:
                        # drains on the software DGE during attention half 1
                        nc.gpsimd.dma_start(
                            out_d[trow, n * 512:(n + 1) * 512],
                            oe[:, n * 512:(n + 1) * 512])
                    elif n == 3:
                        # post-attention: hardware DGE queues are empty
                        nc.sync.dma_start(out_d[trow, 0:1024], oe[:, 0:1024])
                        nc.scalar.dma_start(out_d[trow, 1024:2048],
                                            oe[:, 1024:2048])
                return chunk

            for n in range(4):
                filler.append(mk(n, state))

        def fill(filler, rate):
            """emit floor-accumulated `rate` chunks from the filler queue."""
            filler[0] += rate
            while filler[0] >= 1.0 and len(filler) > 1:
                filler[0] -= 1.0
                filler.pop(1)()

        def drain(filler):
            while len(filler) > 1:
                filler.pop(1)()

        def scores_pair(s, h, kcp, pts):
            """2-bank score tile for key blocks (2kcp, 2kcp+1); one exp.

            Valid because the vision prefix is 256-aligned and the mask is
            all ones: the per-key-partition exp bias vector is identical for
            the two 128-key blocks. Both matmuls are computed full-width;
            super-diagonal garbage (finite) is zeroed after the exp.
            """
            j, p, r = h // 4, h // 2, (h % 2) * 64
            kt = kTp if j * 64 == r else kTs
            kc_e = 2 * kcp
            sp = ps.tile([128, 1024], F32, tag="sp", bufs=2, name="sp")
            for b in (0, 1):
                kc = kc_e + b
                nc.tensor.matmul(
                    sp[:, b * 512:(b + 1) * 512],
                    kt[r:r + 64, kc * 128:(kc + 1) * 128],
                    qTp[p][r:r + 64, s * 512:(s + 1) * 512],
                    start=True, stop=True)
            pt = ptp.tile([128, 1024], BF16, tag="pt", name="pt")
            col = h * 16 + kc_e  # vq=0 bias column (same vector for both)
            nc.scalar.activation(pt[:], sp[:], AF.Exp,
                                 bias=btab[:, col:col + 1], scale=1.0)
            for b in (0, 1):
                kc = kc_e + b
                if kc < 2:  # vision key block: fix up vision-q columns
                    fcol = h * 2 + kc
                    for (c0, c1) in _fixup_runs(qtile_vq, s, 0):
                        nc.vector.tensor_scalar_mul(
                            pt[:, b * 512 + c0:b * 512 + c1],
                            pt[:, b * 512 + c0:b * 512 + c1],
                            ftab[:, fcol:fcol + 1])
            if kc_e >= s * 4:
                # diagonal pair: zero the super-diagonal region, tril the
                # diagonal 128-block, per bank
                for b in (0, 1):
                    dd = (kc_e + b - s * 4) * 128
                    if dd > 0:
                        nc.vector.memset(pt[:, b * 512:b * 512 + dd], 0.0)
                    nc.vector.tensor_mul(pt[:, b * 512 + dd:b * 512 + dd + 128],
                                         pt[:, b * 512 + dd:b * 512 + dd + 128],
                                         trilT[:])
            pts[kc_e] = (pt, 0)
            pts[kc_e + 1] = (pt, 512)

        def pv(s, h, kc, kcmax, yp, pts):
            j = h // 4
            ql = max(0, kc * 128 - s * 512)
            pt, off = pts[kc]
            nc.tensor.matmul(
                yp[:, ql:512], vAall[:, kc * 2 + j, 0:65],
                pt[:, off + ql:off + 512],
                start=(kc == 0), stop=(kc == kcmax - 1),
                skip_group_check=True)
            pts[kc] = None

        def normalize_pair(s, hp, yp0, yp1, ci):
            ya = yap[ci % 4]
            nc.vector.tensor_copy(ya[0:65, 0:512], yp0[:])
            nc.vector.tensor_copy(ya[0:65, 512:1024], yp1[:])
            yaT = p2t.tile([128, 8, 80], BF16, tag="yaT", name="yaT")
            nc.sync.dma_start_transpose(yaT[:], ya[0:80, :])
            ynorm = p2t.tile([128, 512], BF16, tag="ynorm", name="ynorm")
            for qb in range(4):
                rc0 = p2t.tile([128, 1], F32, tag="rc", name="rc0")
                nc.vector.reciprocal(rc0[:], yaT[:, qb, 64:65])
                nc.vector.tensor_scalar_mul(
                    ynorm[:, qb * 128:qb * 128 + 64],
                    yaT[:, qb, 0:64], rc0[:, 0:1])
                rc1 = p2t.tile([128, 1], F32, tag="rc", name="rc1")
                nc.vector.reciprocal(rc1[:], yaT[:, 4 + qb, 64:65])
                nc.vector.tensor_scalar_mul(
                    ynorm[:, qb * 128 + 64:qb * 128 + 128],
                    yaT[:, 4 + qb, 0:64], rc1[:, 0:1])
            nc.sync.dma_start_transpose(
                yT[hp][:, s * 512:(s + 1) * 512].rearrange(
                    "p (b c) -> p b c", b=4), ynorm[:])

        def attention_pair(s, hp, filler, per_kc):
            kcmax = 4 * (s + 1)
            npairs = kcmax // 2
            h0, h1 = 2 * hp, 2 * hp + 1
            yp0 = ps.tile([65, 512], F32, tag="yp0", bufs=1, name="yp0")
            yp1 = ps.tile([65, 512], F32, tag="yp1", bufs=1, name="yp1")
            pts0, pts1 = {}, {}
            scores_pair(s, h0, 0, pts0)
            scores_pair(s, h1, 0, pts1)
            fill(filler, per_kc)
            for kcp in range(npairs):
                for b in (0, 1):
                    kc = 2 * kcp + b
                    pv(s, h0, kc, kcmax, yp0, pts0)
                    pv(s, h1, kc, kcmax, yp1, pts1)
                fill(filler, per_kc)
                if kcp + 1 < npairs:
                    scores_pair(s, h0, kcp + 1, pts0)
                    scores_pair(s, h1, kcp + 1, pts1)
                    fill(filler, per_kc)
            normalize_pair(s, hp, yp0, yp1, s * 4 + hp)

        # --------- emission ------------------------------------------------
        f0 = [0.0]
        proj_chunks(0, f0)               # th0 proj, block-sequential
        drain(f0)

        f1 = [0.0]
        proj_chunks(1, f1)               # th1 proj (24 chunks) rides attn(0)
        for hp in range(4):
            attention_pair(0, hp, f1, per_kc=1.5)   # 16 slots
        drain(f1)

        f2 = [0.0]
        for tb in range(4):              # outproj(0) (16 chunks) rides attn(1)
            outproj_chunks(tb, f2)
        for hp in range(4):
            attention_pair(1, hp, f2, per_kc=0.5)   # 32 slots
        drain(f2)

        f3 = [0.0]
        for tb in range(4, 8):
            outproj_chunks(tb, f3)
        drain(f3)

        for cm in (ost_cm, p2t_cm, ptp_cm, ps_cm, p1x_cm, p1t_cm, p1w_cm,
                   cp_cm):
            cm.__exit__(None, None, None)

    nc.compile()
    return nc


def make_core_inputs(x, cos, sin, attention_mask, is_vision, Wq, Wk, Wv, Wo,
                     gate, b, g):
    cos_b = np.asarray(cos[b], dtype=np.float32)   # [T, 64]
    sin_b = np.asarray(sin[b], dtype=np.float32)
    sgn = np.concatenate([-np.ones(32), np.ones(32)]).astype(np.float32)
    cosT = np.tile(cos_b.T, (2, 1))                            # [128, T]
    sinT = np.tile(sin_b.T * sgn[:, None], (2, 1))             # [128, T]
    vk = np.asarray(is_vision[b], dtype=np.int32)
    maskneg = np.where(np.asarray(attention_mask[b]) > 0, 0.0, NEG)

    hq0 = QH * g
    btab = np.empty((128, 128), dtype=np.float32)
    ftab = np.zeros((128, 16), dtype=np.float32)
    for h in range(QH):
        for vq in range(2):
            for kc in range(8):
                col = h * 16 + vq * 8 + kc
                ks = slice(kc * 128, (kc + 1) * 128)
                btab[:, col] = gate[hq0 + h, 2 * vq + vk[ks]] + maskneg[ks]
        for kc in range(2):
            ks = slice(kc * 128, (kc + 1) * 128)
            ftab[:, h * 2 + kc] = np.exp(
                gate[hq0 + h, 2 + vk[ks]] - gate[hq0 + h, vk[ks]])

    return {
        "xT": np.ascontiguousarray(x[b].T).astype(np.float16),
        "wqT": np.ascontiguousarray(
            Wq[hq0 * 64:hq0 * 64 + 512, :].T * 0.125).astype(np.float16),
        "wkvT": np.ascontiguousarray(
            np.concatenate([Wk[128 * g:128 * g + 128, :].T,
                            Wv[128 * g:128 * g + 128, :].T],
                           axis=1)).astype(np.float16),
        "woT": np.ascontiguousarray(
            Wo[:, hq0 * 64:hq0 * 64 + 512].T).astype(BF),
        "cosT": np.ascontiguousarray(cosT),
        "sinT": np.ascontiguousarray(sinT),
        "btab": btab,
        "ftab": ftab,
        "trilT": (np.arange(128)[:, None] <= np.arange(128)[None, :]
                  ).astype(BF),
        "ones16": np.ones((128, 16), dtype=BF),
    }


def kernel(x, cos, sin, attention_mask, is_vision, Wq, Wk, Wv, Wo, gate):
    x = np.asarray(x, dtype=np.float32)
    cos = np.asarray(cos, dtype=np.float32)
    sin = np.asarray(sin, dtype=np.float32)
    attention_mask = np.asarray(attention_mask, dtype=np.float32)
    is_vision = np.asarray(is_vision)
    Wq = np.asarray(Wq, dtype=np.float32)
    Wk = np.asarray(Wk, dtype=np.float32)
    Wv = np.asarray(Wv, dtype=np.float32)
    Wo = np.asarray(Wo, dtype=np.float32)
    gate = np.asarray(gate, dtype=np.float32)

    # q-side vision flag must be constant within each 128-token tile and
    # identical across batches (holds for the fixed vision-prefix data).
    iv = is_vision.astype(np.int32)
    qtile_vq = []
    for qt in range(NTB):
        blk = iv[:, qt * 128:(qt + 1) * 128]
        assert (blk == blk[0, 0]).all(), "is_vision not 128-tile constant"
        qtile_vq.append(int(blk[0, 0]))
    # the paired-key-block exp assumes an all-ones key mask and a vision
    # flag constant within each 256-token key pair-block, and the fixup
    # path assumes vision q-tiles only appear in the first half
    assert np.all(attention_mask > 0), "paired exp needs all-ones mask"
    for kcp in range(4):
        blk = iv[:, kcp * 256:(kcp + 1) * 256]
        assert (blk == blk[0, 0]).all(), "is_vision not 256-block constant"
    assert not any(qtile_vq[4:]), "vision q-tiles in second half"

    in_maps = [
        make_core_inputs(x, cos, sin, attention_mask, is_vision,
                         Wq, Wk, Wv, Wo, gate, b=c // 4, g=c % 4)
        for c in range(NCORES)
    ]

    nc = build_program(qtile_vq)
    trace = bool(int(os.environ.get("NANOVLM_TRACE", "0")))
    if trace:
        results = _run_traced(nc, in_maps)
    else:
        results = run_bass_kernel_spmd(nc, in_maps, list(range(NCORES))).results
    out = np.empty((B, T, C), dtype=np.float32)
    for b in range(B):
        out[b] = sum(np.asarray(results[4 * b + g]["out"], dtype=np.float32)
                     for g in range(4))
    return out


def _ensure_ntff_hook():
    """The agent image's antenv lacks axon_hooks; shim it and register the
    ctypes NTFF profile hook against the axon PJRT .so."""
    try:
        from antenv.axon_hooks import get_axon_ntff_profile_hook  # noqa: F401
        return True
    except ImportError:
        pass
    import types

    import antenv

    mod = types.ModuleType("antenv.axon_hooks")
    mod._hook = None

    def set_axon_ntff_profile_hook(h):
        mod._hook = h

    def get_axon_ntff_profile_hook():
        return mod._hook

    mod.set_axon_ntff_profile_hook = set_axon_ntff_profile_hook
    mod.get_axon_ntff_profile_hook = get_axon_ntff_profile_hook
    sys.modules["antenv.axon_hooks"] = mod
    antenv.axon_hooks = mod
    if "/root/.axon_site" not in sys.path:
        sys.path.insert(0, "/root/.axon_site")
    try:
        from trn_agent_boot.trn_boot import _ntff_profile_via_ctypes

        hook = _ntff_profile_via_ctypes("/opt/axon/libaxon_pjrt.so")
    except Exception as e:
        print("ntff hook setup failed:", e)
        return False
    if hook is None:
        return False
    set_axon_ntff_profile_hook(hook)
    return True


def _run_traced(nc, in_maps, trace_core=0):
    import glob
    import tempfile

    from concourse import bass2jax
    from concourse._compat import FishPath
    import gauge.profiler

    if not _ensure_ntff_hook():
        print("no NTFF hook; running untraced")
        return run_bass_kernel_spmd(nc, in_maps, list(range(NCORES))).results

    from antenv.axon_hooks import get_axon_ntff_profile_hook

    hook = get_axon_ntff_profile_hook()
    tmpdir = tempfile.mkdtemp(prefix="nanovlm_prof_")
    with hook(tmpdir, [trace_core]):
        results = bass2jax.run_bass_via_pjrt(nc, in_maps, n_cores=NCORES)
    ntffs = glob.glob(os.path.join(tmpdir, "*_body*.ntff"))
    if not ntffs:
        print("no NTFF produced; files:", os.listdir(tmpdir))
        return results
    profile = gauge.profiler.Profile(
        profile_path=FishPath(tmpdir),
        kernel_dev_mode=True,
        profile_on_exit=False,
        bass_kernel=nc.m,
        offline_processing=True,
        fname="*_body*",
    )
    try:
        pr = profile.to_perfetto(model_index=(trace_core,))
        kernel.last_exec_time_ns = pr[0].exec_time_ns
        kernel.last_trace = pr[0].trace_path
        print(f"HW exec time: {pr[0].exec_time_ns} ns")
        print("trace:", pr[0].trace_path)
    except Exception as e:
        print("perfetto conversion failed:", type(e).__name__, e)
        print("ntff dir:", tmpdir)
    return results


# revision 26
# speedup vs baseline: 1.1166x; 1.1166x over previous
"""NanoVLM GQA attention block on 8 Trainium2 NeuronCores.

Sharding: core c = 4*b + g handles batch b (of 2) and head-group g (of 4):
8 q-heads (global 8g..8g+8) and 2 kv-heads (2g, 2g+1). Each core computes a
partial output restricted to its heads' columns of Wo; the host sums the 4
partials per batch (the tensor-parallel reduce, done host-side).

v4: 16-bit datapath (fp16 for x/weights/q/k, bf16 downstream of the ACT
engine), XBAR DMA transposes, causal mask as post-exp tril multiply.

All engine queues are in-order, so emission order is the schedule. The
attention softmax (ACT exp, ~0.7us per tile) paces attention pairs while
the PE only has ~40% utilization there; conversely projection/out-proj are
pure PE. v4 therefore interleaves at matmul-chunk granularity: a filler
queue holds the PE-heavy work (th1 projection during attention half 0,
out-proj of half 0 during attention half 1) and is drained a chunk at a
time inside each pair's kc loop, so the PE queue always holds exp-independent
work behind the attention matmuls. PSUM: 2 rotating proj banks + 4 rotating
score/out-proj banks + 2 pinned y-accumulators. The per-(h,kc) exp is a
single ACT instruction (vq=0 bias); mixed vision/text q-ranges get a cheap
per-partition DVE fixup multiply instead of a second exp.
"""

import os
import sys

sys.path.insert(0, "/opt/trn_rl_repo")

import numpy as np
import ml_dtypes

import concourse.bacc as bacc
import concourse.mybir as mybir
import concourse.tile as tile
from concourse.bass_utils import run_bass_kernel_spmd

F32 = mybir.dt.float32
F16 = mybir.dt.float16
BF16 = mybir.dt.bfloat16
AF = mybir.ActivationFunctionType
ALU = mybir.AluOpType
BF = ml_dtypes.bfloat16

B, T, C = 2, 1024, 2048
NH, NKV, HD = 32, 8, 64
QH, KVH = 8, 2          # per-core q-heads / kv-heads
NTB = T // 128          # 8 t-blocks
NCORES = 8
NEG = -1e30


def _fixup_runs(qtile_vq, s, ql):
    """[c0,c1) column runs with vq==1 inside [ql,512) of an s-half tile."""
    runs, c = [], ql
    while c < 512:
        if qtile_vq[s * 4 + c // 128]:
            ce = c
            while ce < 512 and qtile_vq[s * 4 + ce // 128]:
                ce += 128
            runs.append((c, ce))
            c = ce
        else:
            c += 128
    return runs


def build_program(qtile_vq):
    """qtile_vq: per 128-token q-tile, the is_vision value (0/1), len 8."""
    nc = bacc.Bacc("TRN2", target_bir_lowering=False, debug=False,
                   num_devices=NCORES)

    xT_d = nc.dram_tensor("xT", [C, T], F16, kind="ExternalInput").ap()
    wq_d = nc.dram_tensor("wqT", [C, 512], F16, kind="ExternalInput").ap()
    wkv_d = nc.dram_tensor("wkvT", [C, 256], F16, kind="ExternalInput").ap()
    wo_d = nc.dram_tensor("woT", [512, C], BF16, kind="ExternalInput").ap()
    cos_d = nc.dram_tensor("cosT", [128, T], F32, kind="ExternalInput").ap()
    sin_d = nc.dram_tensor("sinT", [128, T], F32, kind="ExternalInput").ap()
    btab_d = nc.dram_tensor("btab", [128, 128], F32, kind="ExternalInput").ap()
    ftab_d = nc.dram_tensor("ftab", [128, 16], F32, kind="ExternalInput").ap()
    tril_d = nc.dram_tensor("trilT", [128, 128], BF16, kind="ExternalInput").ap()
    ones_d = nc.dram_tensor("ones16", [128, 16], BF16, kind="ExternalInput").ap()
    out_d = nc.dram_tensor("out", [T, C], BF16, kind="ExternalOutput").ap()

    with tile.TileContext(nc) as tc:
        cp_cm = tc.tile_pool(name="const", bufs=1)
        cp = cp_cm.__enter__()
        qTp = [cp.tile([128, T], F16, tag=f"qTp{p}", name=f"qTp{p}")
               for p in range(4)]
        kTp = cp.tile([128, T], F16, tag="kTp")
        kTs = cp.tile([128, T], F16, tag="kTs")  # kv heads swapped
        # [t-part, (tb,j) block, 66]: cols 0:64 v data, col 64 ones
        vAall = cp.tile([128, 16, 66], BF16, tag="vAall")
        vT = [cp.tile([128, 4, 64], BF16, tag=f"vT{i}", name=f"vT{i}")
              for i in range(4)]
        # y_aug evac pads: rows 0:65 used, 65:80 zero filler for the XBAR
        # transpose; declared [96,T] so the filler memset starts at the
        # 32-aligned partition 64 (row 64 is rewritten by every evac)
        yap = [cp.tile([96, T], BF16, tag=f"yap{i}", name=f"yap{i}")
               for i in range(4)]
        yT = [cp.tile([128, T], BF16, tag=f"yT{p}", name=f"yTt{p}")
              for p in range(4)]
        btab = cp.tile([128, 128], F32, tag="btab")
        ftab = cp.tile([128, 16], F32, tag="ftab")
        trilT = cp.tile([128, 128], BF16, tag="trilT")
        cosT = cp.tile([128, T], F32, tag="cosT")
        sinT = cp.tile([128, T], F32, tag="sinT")

        # --------- pools ---------------------------------------------------
        p1w_cm = tc.tile_pool(name="p1w", bufs=1, side="right")
        p1w = p1w_cm.__enter__()
        p1t_cm = tc.tile_pool(name="p1t", bufs=2, side="right")
        p1t = p1t_cm.__enter__()
        p1x_cm = tc.tile_pool(name="p1x", bufs=1, side="right")
        p1x = p1x_cm.__enter__()
        ps_cm = tc.tile_pool(name="ps", bufs=1, space="PSUM")
        ps = ps_cm.__enter__()
        ptp_cm = tc.tile_pool(name="ptp", bufs=8)
        ptp = ptp_cm.__enter__()
        p2t_cm = tc.tile_pool(name="p2t", bufs=4)
        p2t = p2t_cm.__enter__()
        ost_cm = tc.tile_pool(name="ost", bufs=2, side="right")
        ost = ost_cm.__enter__()

        # --------- input DMAs (queue order matters per engine) -------------
        xa, xb, wqs, wkvs = [], [], [], []
        for i in range(16):
            kt = p1w.tile([128, 256], F16, tag=f"wkv{i}", name=f"wkv{i}")
            nc.scalar.dma_start(kt[:], wkv_d[i * 128:(i + 1) * 128, :])
            wkvs.append(kt)
            xt = p1x.tile([128, 512], F16, tag=f"xa{i}", name=f"xa{i}")
            nc.sync.dma_start(xt[:], xT_d[i * 128:(i + 1) * 128, 0:512])
            xa.append(xt)
            wt = p1w.tile([128, 512], F16, tag=f"wq{i}", name=f"wq{i}")
            nc.gpsimd.dma_start(wt[:], wq_d[i * 128:(i + 1) * 128, :])
            wqs.append(wt)
        nc.scalar.dma_start(cosT[:], cos_d)
        nc.scalar.dma_start(sinT[:], sin_d)
        nc.scalar.dma_start(btab[:], btab_d)
        nc.scalar.dma_start(ftab[:], ftab_d)
        nc.scalar.dma_start(trilT[:], tril_d)
        wo = []
        for p in range(4):
            t = p1w.tile([128, C], BF16, tag=f"wo{p}", name=f"wo{p}")
            nc.scalar.dma_start(t[:], wo_d[p * 128:(p + 1) * 128, :])
            wo.append(t)
        for i in range(16):
            xt = p1x.tile([128, 512], F16, tag=f"xb{i}", name=f"xb{i}")
            nc.sync.dma_start(xt[:], xT_d[i * 128:(i + 1) * 128, 512:1024])
            xb.append(xt)
        # late gpsimd setup (after weight DMA issues): ones col + yap floor
        nc.gpsimd.dma_start(vAall[:, :, 64:65], ones_d)
        for i in range(4):
            nc.gpsimd.memset(yap[i][64:96, :], 0.0)

        def rope_blk(pp, blk, th):
            """pp: [128,512] psum with d-major proj; writes qTp/kTp th-slice."""
            tsl = slice(th * 512, (th + 1) * 512)
            dstT = qTp[blk] if blk < 4 else kTp
            # cos is 32-periodic along d and sinT carries the rotate-half
            # sign, so q*cos + rot(q)*sin == u - swap32(v) with u = q*cos,
            # v = q*sinT — both straight PSUM reads, no ACT evacuation
            u = p1t.tile([128, 512], F32, tag="t1", name="u")
            nc.vector.tensor_mul(u[:], pp[:], cosT[:, tsl])
            v = p1t.tile([128, 512], BF16, tag="t2", name="v")
            nc.vector.tensor_mul(v[:], pp[:], sinT[:, tsl])
            vs = p1t.tile([128, 512], BF16, tag="rot", name="vs")
            for q0 in (0, 64):
                nc.gpsimd.dma_start(vs[q0:q0 + 32, :], v[q0 + 32:q0 + 64, :])
                nc.gpsimd.dma_start(vs[q0 + 32:q0 + 64, :], v[q0:q0 + 32, :])
            nc.vector.tensor_sub(dstT[:, tsl], u[:], vs[:])
            if blk == 4:
                nc.gpsimd.dma_start(kTs[0:64, tsl], kTp[64:128, tsl])
                nc.gpsimd.dma_start(kTs[64:128, tsl], kTp[0:64, tsl])

        def v_path(th, pp):
            """evac v proj psum + transpose to t-major vAall slices."""
            vsb = p1t.tile([128, 512], BF16, tag="vsb", name="vsb")
            nc.scalar.copy(vsb[:], pp[:])
            for j in range(KVH):
                vt = vT[th * 2 + j]
                nc.sync.dma_start_transpose(vt[:], vsb[j * 64:(j + 1) * 64, :])
                for qb in range(4):
                    tb = th * 4 + qb
                    nc.vector.tensor_copy(vAall[:, tb * 2 + j, 0:64],
                                          vt[:, qb, :])

        def proj_chunks(th, filler):
            """queue one th's projection as 4-ci chunks on the filler."""
            xs = xa if th == 0 else xb
            for blk in (4, 0, 5, 1, 2, 3):
                state = {}

                def mk(blk, c0, state):
                    def chunk():
                        if c0 == 0:
                            state["pp"] = ps.tile(
                                [128, 512], F32, tag="pp", bufs=2,
                                name=f"pp{th}_{blk}")
                        pp = state["pp"]
                        for ci in range(c0, c0 + 4):
                            if blk < 4:
                                lhsT = wqs[ci][:, blk * 128:(blk + 1) * 128]
                            else:
                                lhsT = wkvs[ci][:, (blk - 4) * 128:(blk - 3) * 128]
                            nc.tensor.matmul(pp[:], lhsT, xs[ci][:],
                                             start=(ci == 0), stop=(ci == 15))
                        if c0 == 12:
                            if blk == 5:
                                v_path(th, pp)
                            else:
                                rope_blk(pp, blk, th)
                    return chunk

                for c0 in (0, 4, 8, 12):
                    filler.append(mk(blk, c0, state))

        def outproj_chunks(tb, filler):
            """queue one out-proj t-block as per-n chunks on the filler."""
            trow = slice(tb * 128, (tb + 1) * 128)
            state = {}

            def mk(n, state):
                def chunk():
                    if n == 0:
                        state["oe"] = ost.tile([128, C], BF16, tag="oe",
                                               name="oe")
                    oe = state["oe"]
                    op = ps.tile([128, 512], F32, tag="pp", bufs=2, name="op")
                    for p in range(4):
                        nc.tensor.matmul(
                            op[:], yT[p][:, trow],
                            wo[p][:, n * 512:(n + 1) * 512],
                            start=(p == 0), stop=(p == 3))
                    nc.vector.tensor_copy(oe[:, n * 512:(n + 1) * 512], op[:])
                    if tb < 4:
                        # drains on the software DGE during attention half 1
                        nc.gpsimd.dma_start(
                            out_d[trow, n * 512:(n + 1) * 512],
                            oe[:, n * 512:(n + 1) * 512])
                    elif n == 3:
                        # post-attention: hardware DGE queues are empty
                        nc.sync.dma_start(out_d[trow, 0:1024], oe[:, 0:1024])
                        nc.scalar.dma_start(out_d[trow, 1024:2048],
                                            oe[:, 1024:2048])
                return chunk

            for n in range(4):
                filler.append(mk(n, state))

        def fill(filler, rate):
            """emit floor-accumulated `rate` chunks from the filler queue."""
            filler[0] += rate
            while filler[0] >= 1.0 and len(filler) > 1:
                filler[0] -= 1.0
                filler.pop(1)()

        def drain(filler):
            while len(filler) > 1:
                filler.pop(1)()

        def scores(s, h, kc, pts):
            j, p, r = h // 4, h // 2, (h % 2) * 64
            kt = kTp if j * 64 == r else kTs
            ql = max(0, kc * 128 - s * 512)
            sp = ps.tile([128, 512], F32, tag="sp", bufs=4, name="sp")
            nc.tensor.matmul(
                sp[:, ql:512],
                kt[r:r + 64, kc * 128:(kc + 1) * 128],
                qTp[p][r:r + 64, s * 512 + ql:(s + 1) * 512],
                start=True, stop=True)
            pt = ptp.tile([128, 512], BF16, tag="pt", name="pt")
            col = h * 16 + kc  # vq=0 bias column
            nc.scalar.activation(pt[:, ql:512], sp[:, ql:512], AF.Exp,
                                 bias=btab[:, col:col + 1], scale=1.0)
            for (c0, c1) in _fixup_runs(qtile_vq, s, ql):
                fcol = h * 2 + kc  # only kc<2 can be mixed here
                nc.vector.tensor_scalar_mul(pt[:, c0:c1], pt[:, c0:c1],
                                            ftab[:, fcol:fcol + 1])
            if s * 4 <= kc < s * 4 + 4:
                # causal diagonal block: zero upper triangle post-exp
                dc = kc * 128 - s * 512
                nc.vector.tensor_mul(pt[:, dc:dc + 128],
                                     pt[:, dc:dc + 128], trilT[:])
            pts[kc] = pt

        def pv(s, h, kc, kcmax, yp, pts):
            j = h // 4
            ql = max(0, kc * 128 - s * 512)
            nc.tensor.matmul(
                yp[:, ql:512], vAall[:, kc * 2 + j, 0:65], pts[kc][:, ql:512],
                start=(kc == 0), stop=(kc == kcmax - 1),
                skip_group_check=True)
            pts[kc] = None

        def normalize_pair(s, hp, yp0, yp1, ci):
            ya = yap[ci % 4]
            nc.vector.tensor_copy(ya[0:65, 0:512], yp0[:])
            nc.vector.tensor_copy(ya[0:65, 512:1024], yp1[:])
            yaT = p2t.tile([128, 8, 80], BF16, tag="yaT", name="yaT")
            nc.sync.dma_start_transpose(yaT[:], ya[0:80, :])
            ynorm = p2t.tile([128, 512], BF16, tag="ynorm", name="ynorm")
            for qb in range(4):
                rc0 = p2t.tile([128, 1], F32, tag="rc", name="rc0")
                nc.vector.reciprocal(rc0[:], yaT[:, qb, 64:65])
                nc.vector.tensor_scalar_mul(
                    ynorm[:, qb * 128:qb * 128 + 64],
                    yaT[:, qb, 0:64], rc0[:, 0:1])
                rc1 = p2t.tile([128, 1], F32, tag="rc", name="rc1")
                nc.vector.reciprocal(rc1[:], yaT[:, 4 + qb, 64:65])
                nc.vector.tensor_scalar_mul(
                    ynorm[:, qb * 128 + 64:qb * 128 + 128],
                    yaT[:, 4 + qb, 0:64], rc1[:, 0:1])
            nc.sync.dma_start_transpose(
                yT[hp][:, s * 512:(s + 1) * 512].rearrange(
                    "p (b c) -> p b c", b=4), ynorm[:])

        def attention_pair(s, hp, filler, per_kc):
            kcmax = 4 * (s + 1)
            h0, h1 = 2 * hp, 2 * hp + 1
            yp0 = ps.tile([65, 512], F32, tag="yp0", bufs=1, name="yp0")
            yp1 = ps.tile([65, 512], F32, tag="yp1", bufs=1, name="yp1")
            pts0, pts1 = {}, {}
            for k in range(min(2, kcmax)):
                scores(s, h0, k, pts0)
                scores(s, h1, k, pts1)
                fill(filler, per_kc)
            for kc in range(kcmax):
                if kc + 2 < kcmax:
                    scores(s, h0, kc + 2, pts0)
                    scores(s, h1, kc + 2, pts1)
                pv(s, h0, kc, kcmax, yp0, pts0)
                pv(s, h1, kc, kcmax, yp1, pts1)
                fill(filler, per_kc)
            normalize_pair(s, hp, yp0, yp1, s * 4 + hp)

        # --------- emission ------------------------------------------------
        f0 = [0.0]
        proj_chunks(0, f0)               # th0 proj, block-sequential
        drain(f0)

        # attn(0) is PE-rich (ACT has slack): consume only enough th1-proj
        # chunks to plug its ACT deficit; the rest shifts into ACT-bound
        # attn(1) so the PE queue never drains there.
        f1 = [0.0]
        proj_chunks(1, f1)               # th1 proj: 24 chunks
        for hp in range(4):
            attention_pair(0, hp, f1, per_kc=0.45)  # 24 slots -> ~11 chunks
        for tb in range(4):              # outproj(0): 16 chunks
            outproj_chunks(tb, f1)
        for hp in range(4):
            attention_pair(1, hp, f1, per_kc=0.73)  # 40 slots -> rest
        drain(f1)

        f3 = [0.0]
        for tb in range(4, 8):
            outproj_chunks(tb, f3)
        drain(f3)

        for cm in (ost_cm, p2t_cm, ptp_cm, ps_cm, p1x_cm, p1t_cm, p1w_cm,
                   cp_cm):
            cm.__exit__(None, None, None)

    nc.compile()
    return nc


def make_core_inputs(x, cos, sin, attention_mask, is_vision, Wq, Wk, Wv, Wo,
                     gate, b, g):
    cos_b = np.asarray(cos[b], dtype=np.float32)   # [T, 64]
    sin_b = np.asarray(sin[b], dtype=np.float32)
    sgn = np.concatenate([-np.ones(32), np.ones(32)]).astype(np.float32)
    cosT = np.tile(cos_b.T, (2, 1))                            # [128, T]
    sinT = np.tile(sin_b.T * sgn[:, None], (2, 1))             # [128, T]
    vk = np.asarray(is_vision[b], dtype=np.int32)
    maskneg = np.where(np.asarray(attention_mask[b]) > 0, 0.0, NEG)

    hq0 = QH * g
    btab = np.empty((128, 128), dtype=np.float32)
    ftab = np.zeros((128, 16), dtype=np.float32)
    for h in range(QH):
        for vq in range(2):
            for kc in range(8):
                col = h * 16 + vq * 8 + kc
                ks = slice(kc * 128, (kc + 1) * 128)
                btab[:, col] = gate[hq0 + h, 2 * vq + vk[ks]] + maskneg[ks]
        for kc in range(2):
            ks = slice(kc * 128, (kc + 1) * 128)
            ftab[:, h * 2 + kc] = np.exp(
                gate[hq0 + h, 2 + vk[ks]] - gate[hq0 + h, vk[ks]])

    return {
        "xT": np.ascontiguousarray(x[b].T).astype(np.float16),
        "wqT": np.ascontiguousarray(
            Wq[hq0 * 64:hq0 * 64 + 512, :].T * 0.125).astype(np.float16),
        "wkvT": np.ascontiguousarray(
            np.concatenate([Wk[128 * g:128 * g + 128, :].T,
                            Wv[128 * g:128 * g + 128, :].T],
                           axis=1)).astype(np.float16),
        "woT": np.ascontiguousarray(
            Wo[:, hq0 * 64:hq0 * 64 + 512].T).astype(BF),
        "cosT": np.ascontiguousarray(cosT),
        "sinT": np.ascontiguousarray(sinT),
        "btab": btab,
        "ftab": ftab,
        "trilT": (np.arange(128)[:, None] <= np.arange(128)[None, :]
                  ).astype(BF),
        "ones16": np.ones((128, 16), dtype=BF),
    }


def kernel(x, cos, sin, attention_mask, is_vision, Wq, Wk, Wv, Wo, gate):
    x = np.asarray(x, dtype=np.float32)
    cos = np.asarray(cos, dtype=np.float32)
    sin = np.asarray(sin, dtype=np.float32)
    attention_mask = np.asarray(attention_mask, dtype=np.float32)
    is_vision = np.asarray(is_vision)
    Wq = np.asarray(Wq, dtype=np.float32)
    Wk = np.asarray(Wk, dtype=np.float32)
    Wv = np.asarray(Wv, dtype=np.float32)
    Wo = np.asarray(Wo, dtype=np.float32)
    gate = np.asarray(gate, dtype=np.float32)

    # q-side vision flag must be constant within each 128-token tile and
    # identical across batches (holds for the fixed vision-prefix data).
    iv = is_vision.astype(np.int32)
    qtile_vq = []
    for qt in range(NTB):
        blk = iv[:, qt * 128:(qt + 1) * 128]
        assert (blk == blk[0, 0]).all(), "is_vision not 128-tile constant"
        qtile_vq.append(int(blk[0, 0]))
    # the paired-key-block exp assumes an all-ones key mask and a vision
    # flag constant within each 256-token key pair-block, and the fixup
    # path assumes vision q-tiles only appear in the first half
    assert np.all(attention_mask > 0), "paired exp needs all-ones mask"
    for kcp in range(4):
        blk = iv[:, kcp * 256:(kcp + 1) * 256]
        assert (blk == blk[0, 0]).all(), "is_vision not 256-block constant"
    assert not any(qtile_vq[4:]), "vision q-tiles in second half"

    in_maps = [
        make_core_inputs(x, cos, sin, attention_mask, is_vision,
                         Wq, Wk, Wv, Wo, gate, b=c // 4, g=c % 4)
        for c in range(NCORES)
    ]

    nc = build_program(qtile_vq)
    trace = bool(int(os.environ.get("NANOVLM_TRACE", "0")))
    if trace:
        results = _run_traced(nc, in_maps)
    else:
        results = run_bass_kernel_spmd(nc, in_maps, list(range(NCORES))).results
    out = np.empty((B, T, C), dtype=np.float32)
    for b in range(B):
        out[b] = sum(np.asarray(results[4 * b + g]["out"], dtype=np.float32)
                     for g in range(4))
    return out


def _ensure_ntff_hook():
    """The agent image's antenv lacks axon_hooks; shim it and register the
    ctypes NTFF profile hook against the axon PJRT .so."""
    try:
        from antenv.axon_hooks import get_axon_ntff_profile_hook  # noqa: F401
        return True
    except ImportError:
        pass
    import types

    import antenv

    mod = types.ModuleType("antenv.axon_hooks")
    mod._hook = None

    def set_axon_ntff_profile_hook(h):
        mod._hook = h

    def get_axon_ntff_profile_hook():
        return mod._hook

    mod.set_axon_ntff_profile_hook = set_axon_ntff_profile_hook
    mod.get_axon_ntff_profile_hook = get_axon_ntff_profile_hook
    sys.modules["antenv.axon_hooks"] = mod
    antenv.axon_hooks = mod
    if "/root/.axon_site" not in sys.path:
        sys.path.insert(0, "/root/.axon_site")
    try:
        from trn_agent_boot.trn_boot import _ntff_profile_via_ctypes

        hook = _ntff_profile_via_ctypes("/opt/axon/libaxon_pjrt.so")
    except Exception as e:
        print("ntff hook setup failed:", e)
        return False
    if hook is None:
        return False
    set_axon_ntff_profile_hook(hook)
    return True


def _run_traced(nc, in_maps, trace_core=0):
    import glob
    import tempfile

    from concourse import bass2jax
    from concourse._compat import FishPath
    import gauge.profiler

    if not _ensure_ntff_hook():
        print("no NTFF hook; running untraced")
        return run_bass_kernel_spmd(nc, in_maps, list(range(NCORES))).results

    from antenv.axon_hooks import get_axon_ntff_profile_hook

    hook = get_axon_ntff_profile_hook()
    tmpdir = tempfile.mkdtemp(prefix="nanovlm_prof_")
    with hook(tmpdir, [trace_core]):
        results = bass2jax.run_bass_via_pjrt(nc, in_maps, n_cores=NCORES)
    ntffs = glob.glob(os.path.join(tmpdir, "*_body*.ntff"))
    if not ntffs:
        print("no NTFF produced; files:", os.listdir(tmpdir))
        return results
    profile = gauge.profiler.Profile(
        profile_path=FishPath(tmpdir),
        kernel_dev_mode=True,
        profile_on_exit=False,
        bass_kernel=nc.m,
        offline_processing=True,
        fname="*_body*",
    )
    try:
        pr = profile.to_perfetto(model_index=(trace_core,))
        kernel.last_exec_time_ns = pr[0].exec_time_ns
        kernel.last_trace = pr[0].trace_path
        print(f"HW exec time: {pr[0].exec_time_ns} ns")
        print("trace:", pr[0].trace_path)
    except Exception as e:
        print("perfetto conversion failed:", type(e).__name__, e)
        print("ntff dir:", tmpdir)
    return results


# revision 28
# speedup vs baseline: 1.1182x; 1.0015x over previous
"""NanoVLM GQA attention block on 8 Trainium2 NeuronCores.

Sharding: core c = 4*b + g handles batch b (of 2) and head-group g (of 4):
8 q-heads (global 8g..8g+8) and 2 kv-heads (2g, 2g+1). Each core computes a
partial output restricted to its heads' columns of Wo; the host sums the 4
partials per batch (the tensor-parallel reduce, done host-side).

v4: 16-bit datapath (fp16 for x/weights/q/k, bf16 downstream of the ACT
engine), XBAR DMA transposes, causal mask as post-exp tril multiply.

All engine queues are in-order, so emission order is the schedule. The
attention softmax (ACT exp, ~0.7us per tile) paces attention pairs while
the PE only has ~40% utilization there; conversely projection/out-proj are
pure PE. v4 therefore interleaves at matmul-chunk granularity: a filler
queue holds the PE-heavy work (th1 projection during attention half 0,
out-proj of half 0 during attention half 1) and is drained a chunk at a
time inside each pair's kc loop, so the PE queue always holds exp-independent
work behind the attention matmuls. PSUM: 2 rotating proj banks + 4 rotating
score/out-proj banks + 2 pinned y-accumulators. The per-(h,kc) exp is a
single ACT instruction (vq=0 bias); mixed vision/text q-ranges get a cheap
per-partition DVE fixup multiply instead of a second exp.
"""

import os
import sys

sys.path.insert(0, "/opt/trn_rl_repo")

import numpy as np
import ml_dtypes

import concourse.bacc as bacc
import concourse.mybir as mybir
import concourse.tile as tile
from concourse.bass_utils import run_bass_kernel_spmd

F32 = mybir.dt.float32
F16 = mybir.dt.float16
BF16 = mybir.dt.bfloat16
AF = mybir.ActivationFunctionType
ALU = mybir.AluOpType
BF = ml_dtypes.bfloat16

B, T, C = 2, 1024, 2048
NH, NKV, HD = 32, 8, 64
QH, KVH = 8, 2          # per-core q-heads / kv-heads
NTB = T // 128          # 8 t-blocks
NCORES = 8
NEG = -1e30


def _fixup_runs(qtile_vq, s, ql):
    """[c0,c1) column runs with vq==1 inside [ql,512) of an s-half tile."""
    runs, c = [], ql
    while c < 512:
        if qtile_vq[s * 4 + c // 128]:
            ce = c
            while ce < 512 and qtile_vq[s * 4 + ce // 128]:
                ce += 128
            runs.append((c, ce))
            c = ce
        else:
            c += 128
    return runs


def build_program(qtile_vq):
    """qtile_vq: per 128-token q-tile, the is_vision value (0/1), len 8."""
    nc = bacc.Bacc("TRN2", target_bir_lowering=False, debug=False,
                   num_devices=NCORES)

    xT_d = nc.dram_tensor("xT", [C, T], F16, kind="ExternalInput").ap()
    wq_d = nc.dram_tensor("wqT", [C, 512], F16, kind="ExternalInput").ap()
    wkv_d = nc.dram_tensor("wkvT", [C, 256], F16, kind="ExternalInput").ap()
    wo_d = nc.dram_tensor("woT", [512, C], BF16, kind="ExternalInput").ap()
    cos_d = nc.dram_tensor("cosT", [128, T], F32, kind="ExternalInput").ap()
    sin_d = nc.dram_tensor("sinT", [128, T], F32, kind="ExternalInput").ap()
    btab_d = nc.dram_tensor("btab", [128, 128], F32, kind="ExternalInput").ap()
    ftab_d = nc.dram_tensor("ftab", [128, 16], F32, kind="ExternalInput").ap()
    tril_d = nc.dram_tensor("trilT", [128, 128], BF16, kind="ExternalInput").ap()
    ones_d = nc.dram_tensor("ones16", [128, 16], BF16, kind="ExternalInput").ap()
    out_d = nc.dram_tensor("out", [T, C], BF16, kind="ExternalOutput").ap()

    with tile.TileContext(nc) as tc:
        cp_cm = tc.tile_pool(name="const", bufs=1)
        cp = cp_cm.__enter__()
        qTp = [cp.tile([128, T], F16, tag=f"qTp{p}", name=f"qTp{p}")
               for p in range(4)]
        kTp = cp.tile([128, T], F16, tag="kTp")
        kTs = cp.tile([128, T], F16, tag="kTs")  # kv heads swapped
        # [t-part, (tb,j) block, 66]: cols 0:64 v data, col 64 ones
        vAall = cp.tile([128, 16, 66], BF16, tag="vAall")
        vT = [cp.tile([128, 4, 64], BF16, tag=f"vT{i}", name=f"vT{i}")
              for i in range(4)]
        # y_aug evac pads: rows 0:65 used, 65:80 zero filler for the XBAR
        # transpose; declared [96,T] so the filler memset starts at the
        # 32-aligned partition 64 (row 64 is rewritten by every evac)
        yap = [cp.tile([96, T], BF16, tag=f"yap{i}", name=f"yap{i}")
               for i in range(4)]
        yT = [cp.tile([128, T], BF16, tag=f"yT{p}", name=f"yTt{p}")
              for p in range(4)]
        btab = cp.tile([128, 128], F32, tag="btab")
        ftab = cp.tile([128, 16], F32, tag="ftab")
        trilT = cp.tile([128, 128], BF16, tag="trilT")
        cosT = cp.tile([128, T], F32, tag="cosT")
        sinT = cp.tile([128, T], F32, tag="sinT")

        # --------- pools ---------------------------------------------------
        p1w_cm = tc.tile_pool(name="p1w", bufs=1, side="right")
        p1w = p1w_cm.__enter__()
        p1t_cm = tc.tile_pool(name="p1t", bufs=2, side="right")
        p1t = p1t_cm.__enter__()
        p1x_cm = tc.tile_pool(name="p1x", bufs=1, side="right")
        p1x = p1x_cm.__enter__()
        ps_cm = tc.tile_pool(name="ps", bufs=1, space="PSUM")
        ps = ps_cm.__enter__()
        ptp_cm = tc.tile_pool(name="ptp", bufs=10)
        ptp = ptp_cm.__enter__()
        p2t_cm = tc.tile_pool(name="p2t", bufs=4)
        p2t = p2t_cm.__enter__()
        ost_cm = tc.tile_pool(name="ost", bufs=2, side="right")
        ost = ost_cm.__enter__()

        # --------- input DMAs (queue order matters per engine) -------------
        xa, xb, wqs, wkvs = [], [], [], []
        for i in range(16):
            kt = p1w.tile([128, 256], F16, tag=f"wkv{i}", name=f"wkv{i}")
            nc.scalar.dma_start(kt[:], wkv_d[i * 128:(i + 1) * 128, :])
            wkvs.append(kt)
            xt = p1x.tile([128, 512], F16, tag=f"xa{i}", name=f"xa{i}")
            nc.sync.dma_start(xt[:], xT_d[i * 128:(i + 1) * 128, 0:512])
            xa.append(xt)
            wt = p1w.tile([128, 512], F16, tag=f"wq{i}", name=f"wq{i}")
            nc.gpsimd.dma_start(wt[:], wq_d[i * 128:(i + 1) * 128, :])
            wqs.append(wt)
        nc.scalar.dma_start(cosT[:], cos_d)
        nc.scalar.dma_start(sinT[:], sin_d)
        nc.scalar.dma_start(btab[:], btab_d)
        nc.scalar.dma_start(ftab[:], ftab_d)
        nc.scalar.dma_start(trilT[:], tril_d)
        wo = []
        for p in range(4):
            t = p1w.tile([128, C], BF16, tag=f"wo{p}", name=f"wo{p}")
            nc.scalar.dma_start(t[:], wo_d[p * 128:(p + 1) * 128, :])
            wo.append(t)
        for i in range(16):
            xt = p1x.tile([128, 512], F16, tag=f"xb{i}", name=f"xb{i}")
            nc.sync.dma_start(xt[:], xT_d[i * 128:(i + 1) * 128, 512:1024])
            xb.append(xt)
        # late gpsimd setup (after weight DMA issues): ones col + yap floor
        nc.gpsimd.dma_start(vAall[:, :, 64:65], ones_d)
        for i in range(4):
            nc.gpsimd.memset(yap[i][64:96, :], 0.0)

        def rope_blk(pp, blk, th):
            """pp: [128,512] psum with d-major proj; writes qTp/kTp th-slice."""
            tsl = slice(th * 512, (th + 1) * 512)
            dstT = qTp[blk] if blk < 4 else kTp
            # cos is 32-periodic along d and sinT carries the rotate-half
            # sign, so q*cos + rot(q)*sin == u - swap32(v) with u = q*cos,
            # v = q*sinT — both straight PSUM reads, no ACT evacuation
            u = p1t.tile([128, 512], F32, tag="t1", name="u")
            nc.vector.tensor_mul(u[:], pp[:], cosT[:, tsl])
            v = p1t.tile([128, 512], BF16, tag="t2", name="v")
            nc.vector.tensor_mul(v[:], pp[:], sinT[:, tsl])
            vs = p1t.tile([128, 512], BF16, tag="rot", name="vs")
            for q0 in (0, 64):
                nc.gpsimd.dma_start(vs[q0:q0 + 32, :], v[q0 + 32:q0 + 64, :])
                nc.gpsimd.dma_start(vs[q0 + 32:q0 + 64, :], v[q0:q0 + 32, :])
            nc.vector.tensor_sub(dstT[:, tsl], u[:], vs[:])
            if blk == 4:
                nc.gpsimd.dma_start(kTs[0:64, tsl], kTp[64:128, tsl])
                nc.gpsimd.dma_start(kTs[64:128, tsl], kTp[0:64, tsl])

        def v_path(th, pp):
            """evac v proj psum + transpose to t-major vAall slices."""
            vsb = p1t.tile([128, 512], BF16, tag="vsb", name="vsb")
            nc.scalar.copy(vsb[:], pp[:])
            for j in range(KVH):
                vt = vT[th * 2 + j]
                nc.sync.dma_start_transpose(vt[:], vsb[j * 64:(j + 1) * 64, :])
                for qb in range(4):
                    tb = th * 4 + qb
                    nc.vector.tensor_copy(vAall[:, tb * 2 + j, 0:64],
                                          vt[:, qb, :])

        def proj_chunks(th, filler):
            """queue one th's projection as 4-ci chunks on the filler."""
            xs = xa if th == 0 else xb
            for blk in (4, 0, 5, 1, 2, 3):
                state = {}

                def mk(blk, c0, state):
                    def chunk():
                        if c0 == 0:
                            state["pp"] = ps.tile(
                                [128, 512], F32, tag="pp", bufs=2,
                                name=f"pp{th}_{blk}")
                        pp = state["pp"]
                        for ci in range(c0, c0 + 4):
                            if blk < 4:
                                lhsT = wqs[ci][:, blk * 128:(blk + 1) * 128]
                            else:
                                lhsT = wkvs[ci][:, (blk - 4) * 128:(blk - 3) * 128]
                            nc.tensor.matmul(pp[:], lhsT, xs[ci][:],
                                             start=(ci == 0), stop=(ci == 15))
                        if c0 == 12:
                            if blk == 5:
                                v_path(th, pp)
                            else:
                                rope_blk(pp, blk, th)
                    return chunk

                for c0 in (0, 4, 8, 12):
                    filler.append(mk(blk, c0, state))

        def outproj_chunks(tb, filler):
            """queue one out-proj t-block as per-n chunks on the filler."""
            trow = slice(tb * 128, (tb + 1) * 128)
            state = {}

            def mk(n, state):
                def chunk():
                    if n == 0:
                        state["oe"] = ost.tile([128, C], BF16, tag="oe",
                                               name="oe")
                    oe = state["oe"]
                    op = ps.tile([128, 512], F32, tag="pp", bufs=2, name="op")
                    for p in range(4):
                        nc.tensor.matmul(
                            op[:], yT[p][:, trow],
                            wo[p][:, n * 512:(n + 1) * 512],
                            start=(p == 0), stop=(p == 3))
                    nc.vector.tensor_copy(oe[:, n * 512:(n + 1) * 512], op[:])
                    if tb < 4:
                        # drains on the software DGE during attention half 1
                        nc.gpsimd.dma_start(
                            out_d[trow, n * 512:(n + 1) * 512],
                            oe[:, n * 512:(n + 1) * 512])
                    elif n == 3:
                        # post-attention: hardware DGE queues are empty
                        nc.sync.dma_start(out_d[trow, 0:1024], oe[:, 0:1024])
                        nc.scalar.dma_start(out_d[trow, 1024:2048],
                                            oe[:, 1024:2048])
                return chunk

            for n in range(4):
                filler.append(mk(n, state))

        def fill(filler, rate):
            """emit floor-accumulated `rate` chunks from the filler queue."""
            filler[0] += rate
            while filler[0] >= 1.0 and len(filler) > 1:
                filler[0] -= 1.0
                filler.pop(1)()

        def drain(filler):
            while len(filler) > 1:
                filler.pop(1)()

        def scores(s, h, kc, pts):
            j, p, r = h // 4, h // 2, (h % 2) * 64
            kt = kTp if j * 64 == r else kTs
            ql = max(0, kc * 128 - s * 512)
            sp = ps.tile([128, 512], F32, tag="sp", bufs=4, name="sp")
            nc.tensor.matmul(
                sp[:, ql:512],
                kt[r:r + 64, kc * 128:(kc + 1) * 128],
                qTp[p][r:r + 64, s * 512 + ql:(s + 1) * 512],
                start=True, stop=True)
            pt = ptp.tile([128, 512], BF16, tag="pt", name="pt")
            col = h * 16 + kc  # vq=0 bias column
            nc.scalar.activation(pt[:, ql:512], sp[:, ql:512], AF.Exp,
                                 bias=btab[:, col:col + 1], scale=1.0)
            for (c0, c1) in _fixup_runs(qtile_vq, s, ql):
                fcol = h * 2 + kc  # only kc<2 can be mixed here
                nc.vector.tensor_scalar_mul(pt[:, c0:c1], pt[:, c0:c1],
                                            ftab[:, fcol:fcol + 1])
            if s * 4 <= kc < s * 4 + 4:
                # causal diagonal block: zero upper triangle post-exp
                dc = kc * 128 - s * 512
                nc.vector.tensor_mul(pt[:, dc:dc + 128],
                                     pt[:, dc:dc + 128], trilT[:])
            pts[kc] = pt

        def pv(s, h, kc, kcmax, yp, pts):
            j = h // 4
            ql = max(0, kc * 128 - s * 512)
            nc.tensor.matmul(
                yp[:, ql:512], vAall[:, kc * 2 + j, 0:65], pts[kc][:, ql:512],
                start=(kc == 0), stop=(kc == kcmax - 1),
                skip_group_check=True)
            pts[kc] = None

        def normalize_pair(s, hp, yp0, yp1, ci):
            ya = yap[ci % 4]
            nc.vector.tensor_copy(ya[0:65, 0:512], yp0[:])
            nc.vector.tensor_copy(ya[0:65, 512:1024], yp1[:])
            yaT = p2t.tile([128, 8, 80], BF16, tag="yaT", name="yaT")
            nc.sync.dma_start_transpose(yaT[:], ya[0:80, :])
            ynorm = p2t.tile([128, 512], BF16, tag="ynorm", name="ynorm")
            for qb in range(4):
                rc0 = p2t.tile([128, 1], F32, tag="rc", name="rc0")
                nc.vector.reciprocal(rc0[:], yaT[:, qb, 64:65])
                nc.vector.tensor_scalar_mul(
                    ynorm[:, qb * 128:qb * 128 + 64],
                    yaT[:, qb, 0:64], rc0[:, 0:1])
                rc1 = p2t.tile([128, 1], F32, tag="rc", name="rc1")
                nc.vector.reciprocal(rc1[:], yaT[:, 4 + qb, 64:65])
                nc.vector.tensor_scalar_mul(
                    ynorm[:, qb * 128 + 64:qb * 128 + 128],
                    yaT[:, 4 + qb, 0:64], rc1[:, 0:1])
            nc.sync.dma_start_transpose(
                yT[hp][:, s * 512:(s + 1) * 512].rearrange(
                    "p (b c) -> p b c", b=4), ynorm[:])

        def attention_pair(s, hp, filler, per_kc):
            kcmax = 4 * (s + 1)
            h0, h1 = 2 * hp, 2 * hp + 1
            yp0 = ps.tile([65, 512], F32, tag="yp0", bufs=1, name="yp0")
            yp1 = ps.tile([65, 512], F32, tag="yp1", bufs=1, name="yp1")
            pts0, pts1 = {}, {}
            for k in range(min(2, kcmax)):
                scores(s, h0, k, pts0)
                scores(s, h1, k, pts1)
                fill(filler, per_kc)
            for kc in range(kcmax):
                if kc + 2 < kcmax:
                    scores(s, h0, kc + 2, pts0)
                    scores(s, h1, kc + 2, pts1)
                pv(s, h0, kc, kcmax, yp0, pts0)
                pv(s, h1, kc, kcmax, yp1, pts1)
                fill(filler, per_kc)
            normalize_pair(s, hp, yp0, yp1, s * 4 + hp)

        # --------- emission ------------------------------------------------
        f0 = [0.0]
        proj_chunks(0, f0)               # th0 proj, block-sequential
        drain(f0)

        # attn(0) is PE-rich (ACT has slack): consume only enough th1-proj
        # chunks to plug its ACT deficit; the rest shifts into ACT-bound
        # attn(1) so the PE queue never drains there.
        f1 = [0.0]
        proj_chunks(1, f1)               # th1 proj: 24 chunks
        for hp in range(4):
            # 24 slots x 0.5 -> exactly blk4+blk0+blk5 (incl v-path) drain
            # before attention half 1 needs second-half V tiles
            attention_pair(0, hp, f1, per_kc=0.5)
        for tb in range(4):              # outproj(0): 16 chunks
            outproj_chunks(tb, f1)
        for hp in range(4):
            attention_pair(1, hp, f1, per_kc=0.7)   # 40 slots -> rest
        drain(f1)

        f3 = [0.0]
        for tb in range(4, 8):
            outproj_chunks(tb, f3)
        drain(f3)

        for cm in (ost_cm, p2t_cm, ptp_cm, ps_cm, p1x_cm, p1t_cm, p1w_cm,
                   cp_cm):
            cm.__exit__(None, None, None)

    nc.compile()
    return nc


def make_core_inputs(x, cos, sin, attention_mask, is_vision, Wq, Wk, Wv, Wo,
                     gate, b, g):
    cos_b = np.asarray(cos[b], dtype=np.float32)   # [T, 64]
    sin_b = np.asarray(sin[b], dtype=np.float32)
    sgn = np.concatenate([-np.ones(32), np.ones(32)]).astype(np.float32)
    cosT = np.tile(cos_b.T, (2, 1))                            # [128, T]
    sinT = np.tile(sin_b.T * sgn[:, None], (2, 1))             # [128, T]
    vk = np.asarray(is_vision[b], dtype=np.int32)
    maskneg = np.where(np.asarray(attention_mask[b]) > 0, 0.0, NEG)

    hq0 = QH * g
    btab = np.empty((128, 128), dtype=np.float32)
    ftab = np.zeros((128, 16), dtype=np.float32)
    for h in range(QH):
        for vq in range(2):
            for kc in range(8):
                col = h * 16 + vq * 8 + kc
                ks = slice(kc * 128, (kc + 1) * 128)
                btab[:, col] = gate[hq0 + h, 2 * vq + vk[ks]] + maskneg[ks]
        for kc in range(2):
            ks = slice(kc * 128, (kc + 1) * 128)
            ftab[:, h * 2 + kc] = np.exp(
                gate[hq0 + h, 2 + vk[ks]] - gate[hq0 + h, vk[ks]])

    return {
        "xT": np.ascontiguousarray(x[b].T).astype(np.float16),
        "wqT": np.ascontiguousarray(
            Wq[hq0 * 64:hq0 * 64 + 512, :].T * 0.125).astype(np.float16),
        "wkvT": np.ascontiguousarray(
            np.concatenate([Wk[128 * g:128 * g + 128, :].T,
                            Wv[128 * g:128 * g + 128, :].T],
                           axis=1)).astype(np.float16),
        "woT": np.ascontiguousarray(
            Wo[:, hq0 * 64:hq0 * 64 + 512].T).astype(BF),
        "cosT": np.ascontiguousarray(cosT),
        "sinT": np.ascontiguousarray(sinT),
        "btab": btab,
        "ftab": ftab,
        "trilT": (np.arange(128)[:, None] <= np.arange(128)[None, :]
                  ).astype(BF),
        "ones16": np.ones((128, 16), dtype=BF),
    }


def kernel(x, cos, sin, attention_mask, is_vision, Wq, Wk, Wv, Wo, gate):
    x = np.asarray(x, dtype=np.float32)
    cos = np.asarray(cos, dtype=np.float32)
    sin = np.asarray(sin, dtype=np.float32)
    attention_mask = np.asarray(attention_mask, dtype=np.float32)
    is_vision = np.asarray(is_vision)
    Wq = np.asarray(Wq, dtype=np.float32)
    Wk = np.asarray(Wk, dtype=np.float32)
    Wv = np.asarray(Wv, dtype=np.float32)
    Wo = np.asarray(Wo, dtype=np.float32)
    gate = np.asarray(gate, dtype=np.float32)

    # q-side vision flag must be constant within each 128-token tile and
    # identical across batches (holds for the fixed vision-prefix data).
    iv = is_vision.astype(np.int32)
    qtile_vq = []
    for qt in range(NTB):
        blk = iv[:, qt * 128:(qt + 1) * 128]
        assert (blk == blk[0, 0]).all(), "is_vision not 128-tile constant"
        qtile_vq.append(int(blk[0, 0]))
    # the paired-key-block exp assumes an all-ones key mask and a vision
    # flag constant within each 256-token key pair-block, and the fixup
    # path assumes vision q-tiles only appear in the first half
    assert np.all(attention_mask > 0), "paired exp needs all-ones mask"
    for kcp in range(4):
        blk = iv[:, kcp * 256:(kcp + 1) * 256]
        assert (blk == blk[0, 0]).all(), "is_vision not 256-block constant"
    assert not any(qtile_vq[4:]), "vision q-tiles in second half"

    in_maps = [
        make_core_inputs(x, cos, sin, attention_mask, is_vision,
                         Wq, Wk, Wv, Wo, gate, b=c // 4, g=c % 4)
        for c in range(NCORES)
    ]

    nc = build_program(qtile_vq)
    trace = bool(int(os.environ.get("NANOVLM_TRACE", "0")))
    if trace:
        results = _run_traced(nc, in_maps)
    else:
        results = run_bass_kernel_spmd(nc, in_maps, list(range(NCORES))).results
    out = np.empty((B, T, C), dtype=np.float32)
    for b in range(B):
        out[b] = sum(np.asarray(results[4 * b + g]["out"], dtype=np.float32)
                     for g in range(4))
    return out


def _ensure_ntff_hook():
    """The agent image's antenv lacks axon_hooks; shim it and register the
    ctypes NTFF profile hook against the axon PJRT .so."""
    try:
        from antenv.axon_hooks import get_axon_ntff_profile_hook  # noqa: F401
        return True
    except ImportError:
        pass
    import types

    import antenv

    mod = types.ModuleType("antenv.axon_hooks")
    mod._hook = None

    def set_axon_ntff_profile_hook(h):
        mod._hook = h

    def get_axon_ntff_profile_hook():
        return mod._hook

    mod.set_axon_ntff_profile_hook = set_axon_ntff_profile_hook
    mod.get_axon_ntff_profile_hook = get_axon_ntff_profile_hook
    sys.modules["antenv.axon_hooks"] = mod
    antenv.axon_hooks = mod
    if "/root/.axon_site" not in sys.path:
        sys.path.insert(0, "/root/.axon_site")
    try:
        from trn_agent_boot.trn_boot import _ntff_profile_via_ctypes

        hook = _ntff_profile_via_ctypes("/opt/axon/libaxon_pjrt.so")
    except Exception as e:
        print("ntff hook setup failed:", e)
        return False
    if hook is None:
        return False
    set_axon_ntff_profile_hook(hook)
    return True


def _run_traced(nc, in_maps, trace_core=0):
    import glob
    import tempfile

    from concourse import bass2jax
    from concourse._compat import FishPath
    import gauge.profiler

    if not _ensure_ntff_hook():
        print("no NTFF hook; running untraced")
        return run_bass_kernel_spmd(nc, in_maps, list(range(NCORES))).results

    from antenv.axon_hooks import get_axon_ntff_profile_hook

    hook = get_axon_ntff_profile_hook()
    tmpdir = tempfile.mkdtemp(prefix="nanovlm_prof_")
    with hook(tmpdir, [trace_core]):
        results = bass2jax.run_bass_via_pjrt(nc, in_maps, n_cores=NCORES)
    ntffs = glob.glob(os.path.join(tmpdir, "*_body*.ntff"))
    if not ntffs:
        print("no NTFF produced; files:", os.listdir(tmpdir))
        return results
    profile = gauge.profiler.Profile(
        profile_path=FishPath(tmpdir),
        kernel_dev_mode=True,
        profile_on_exit=False,
        bass_kernel=nc.m,
        offline_processing=True,
        fname="*_body*",
    )
    try:
        pr = profile.to_perfetto(model_index=(trace_core,))
        kernel.last_exec_time_ns = pr[0].exec_time_ns
        kernel.last_trace = pr[0].trace_path
        print(f"HW exec time: {pr[0].exec_time_ns} ns")
        print("trace:", pr[0].trace_path)
    except Exception as e:
        print("perfetto conversion failed:", type(e).__name__, e)
        print("ntff dir:", tmpdir)
    return results
